# revision 25
# baseline (speedup 1.0000x reference)
"""Causal depthwise Conv1d (B=8, T=4096, C=2048, K=4), fp32, on 8 NeuronCores.

Strategy:
  - Batch-parallel across the 8 cores (B == 8, zero communication).
  - Host transposes x to [B, C, T] so channels land on SBUF partitions and
    time on the free dimension; then every DMA is fully contiguous and the
    4 causal taps are free-dim slices.
  - Per 128-channel block, the depthwise conv is computed on the Tensor
    engine as 4 PSUM-accumulating matmuls with diagonal weight matrices:
        psum[c, t] += diag(w_k)[c, c'] @ x[c', t - 3 + k]
    The diagonal lhsT tiles are built on-chip from a Const identity matrix
    scaled per-partition by the weights (vector tensor_scalar).
  - PSUM -> SBUF eviction with the per-channel bias fused (scalar engine
    activation Identity with bias AP), then contiguous DMA store.
  - Host transposes the [B, C, T] result back to [B, T, C].
"""

import os
from contextlib import ExitStack

import numpy as np

import concourse.bacc as bacc
import concourse.bass as bass
import concourse.mybir as mybir
import concourse.tile as tile
from concourse.bass_utils import run_bass_kernel_spmd

B, T, C, K = 8, 4096, 2048, 4
P = 128                 # partitions per channel block
CB = C // P             # 16 channel blocks
TT = 512                # moving-dim (free) tile per matmul / PSUM bank
HALF = 2048             # free elements per PSUM tile (4 banks)
N_CORES = 8

# "fp32" = exact (4 cyc/row), "fp32r" = fast PE mode (1 cyc/row, reduced
# internal precision).  Overridable via env for A/B testing.
MM_DTYPE = os.environ.get("KERNEL_MM_DTYPE", "fp32")

LAST_EXEC_NS = None
LAST_RESULTS = None

_PROGRAM_CACHE = {}
_PROFILING_READY = False


def _setup_profiling():
    """Register the axon NTFF profile hook (the image lacks
    antenv.axon_hooks, so shim it into sys.modules) and neuter the S3
    artifact upload."""
    global _PROFILING_READY
    if _PROFILING_READY:
        return
    import sys
    import types

    if "antenv.axon_hooks" not in sys.modules:
        mod = types.ModuleType("antenv.axon_hooks")
        mod._hook = None

        def set_axon_ntff_profile_hook(h):
            mod._hook = h

        def get_axon_ntff_profile_hook():
            return mod._hook

        mod.set_axon_ntff_profile_hook = set_axon_ntff_profile_hook
        mod.get_axon_ntff_profile_hook = get_axon_ntff_profile_hook
        sys.modules["antenv.axon_hooks"] = mod
        import antenv

        antenv.axon_hooks = mod

    from antenv.axon_hooks import (
        get_axon_ntff_profile_hook,
        set_axon_ntff_profile_hook,
    )

    if get_axon_ntff_profile_hook() is None:
        from trn_agent_boot.trn_boot import _ntff_profile_via_ctypes

        set_axon_ntff_profile_hook(
            _ntff_profile_via_ctypes("/opt/axon/libaxon_pjrt.so")
        )

    import concourse.bass_utils as bu

    bu.upload_artifacts = lambda tmpdir: str(tmpdir)
    _PROFILING_READY = True


def _build_program(mm_dtype: str) -> bass.Bass:
    nc = bacc.Bacc("TRN2", target_bir_lowering=False, debug=False)

    mmdt = mybir.dt.float32r if mm_dtype == "fp32r" else mybir.dt.float32

    x_d = nc.dram_tensor("x", [C, T], mmdt, kind="ExternalInput")
    w_d = nc.dram_tensor("w", [C, K], mybir.dt.float32, kind="ExternalInput")
    b_d = nc.dram_tensor("b", [C, 1], mybir.dt.float32, kind="ExternalInput")
    o_d = nc.dram_tensor("out", [C, T], mybir.dt.float32, kind="ExternalOutput")
    ident_d = nc.inline_tensor(np.eye(P, dtype=np.float32), "ident")

    with tile.TileContext(nc) as tc, ExitStack() as ctx:
        id_pool = ctx.enter_context(tc.tile_pool(name="id", bufs=1))
        x_pool = ctx.enter_context(tc.tile_pool(name="x", bufs=2))
        out_pool = ctx.enter_context(tc.tile_pool(name="o", bufs=2))
        wb_pool = ctx.enter_context(tc.tile_pool(name="wb", bufs=2))
        lhs_pool = ctx.enter_context(tc.tile_pool(name="lhs", bufs=8))
        y_pool = ctx.enter_context(tc.tile_pool(name="y", bufs=2))
        psum_pool = ctx.enter_context(
            tc.tile_pool(name="ps", bufs=2, space="PSUM")
        )

        id_sb = id_pool.tile([P, P], mybir.dt.float32, tag="ident")
        nc.sync.dma_start(id_sb[:], ident_d[:])

        split2 = mm_dtype == "split2"
        v4 = mm_dtype == "v4"
        v5 = mm_dtype == "v5"
        pe_taps = 2 if split2 else (1 if v4 else K)

        if v5:
            # Pure vector-engine pipeline (no PE, no PSUM), exact fp32:
            #   y0 = x0*w0 + bias     (ACT affine)
            #   t  = x1*w1 + y0       (DVE fused MAC)
            #   t  = x2*w2 + t        (DVE fused MAC, in place)
            #   y3 = x3*w3            (ACT affine)
            #   out = t + y3          (GpSimd add)
            for cb in range(CB):
                c0 = cb * P

                w_sb = wb_pool.tile([P, K], mybir.dt.float32, tag="w")
                nc.gpsimd.dma_start(w_sb[:], w_d[c0 : c0 + P, :])
                bias_sb = wb_pool.tile([P, 1], mybir.dt.float32, tag="bias")
                nc.gpsimd.dma_start(bias_sb[:], b_d[c0 : c0 + P, :])

                xt = x_pool.tile([P, T + K - 1], mybir.dt.float32, tag="x")
                nc.vector.memset(xt[:, 0 : K - 1].bitcast(mybir.dt.uint32), 0)
                nc.sync.dma_start(xt[:, K - 1 : T + K - 1], x_d[c0 : c0 + P, :])

                y0 = y_pool.tile([P, T], mybir.dt.float32, tag="y0")
                nc.scalar.activation(
                    y0[:],
                    xt[:, 0:T],
                    mybir.ActivationFunctionType.Identity,
                    bias=bias_sb[:],
                    scale=w_sb[:, 0:1],
                )
                y3 = y_pool.tile([P, T], mybir.dt.float32, tag="y3")
                nc.scalar.activation(
                    y3[:],
                    xt[:, K - 1 : K - 1 + T],
                    mybir.ActivationFunctionType.Identity,
                    bias=0.0,
                    scale=w_sb[:, 3:4],
                )
                out_sb = out_pool.tile([P, T], mybir.dt.float32, tag="o")
                for half in range(T // HALF):
                    h0 = half * HALF
                    sl = slice(h0, h0 + HALF)
                    nc.vector.scalar_tensor_tensor(
                        y0[:, sl],
                        xt[:, h0 + 1 : h0 + 1 + HALF],
                        w_sb[:, 1:2],
                        y0[:, sl],
                        mybir.AluOpType.mult,
                        mybir.AluOpType.add,
                    )
                    nc.vector.scalar_tensor_tensor(
                        y0[:, sl],
                        xt[:, h0 + 2 : h0 + 2 + HALF],
                        w_sb[:, 2:3],
                        y0[:, sl],
                        mybir.AluOpType.mult,
                        mybir.AluOpType.add,
                    )
                    nc.gpsimd.tensor_tensor(
                        out_sb[:, sl], y0[:, sl], y3[:, sl], mybir.AluOpType.add
                    )
                nc.sync.dma_start(o_d[c0 : c0 + P, :], out_sb[:])

        for cb in range(0 if v5 else CB):
            c0 = cb * P

            w_sb = wb_pool.tile([P, K], mybir.dt.float32, tag="w")
            nc.gpsimd.dma_start(w_sb[:], w_d[c0 : c0 + P, :])
            bias_sb = wb_pool.tile([P, 1], mybir.dt.float32, tag="bias")
            nc.gpsimd.dma_start(bias_sb[:], b_d[c0 : c0 + P, :])

            # x tile with K-1 left halo columns (zeros: causal padding).
            xt = x_pool.tile([P, T + K - 1], mmdt, tag="x")
            nc.vector.memset(xt[:, 0 : K - 1].bitcast(mybir.dt.uint32), 0)
            nc.sync.dma_start(xt[:, K - 1 : T + K - 1], x_d[c0 : c0 + P, :])

            # lhsT_k = diag(w[:, k]) built as identity * w_k (per-partition).
            lhs = []
            for k in range(pe_taps):
                lk = lhs_pool.tile([P, P], mmdt, tag="lhs")
                nc.scalar.mul(lk[:], id_sb[:], w_sb[:, k : k + 1])
                lhs.append(lk)

            out_sb = out_pool.tile([P, T], mybir.dt.float32, tag="o")

            if v4:
                # Tap 3 + bias on ACT via its free affine: y3 = x3*w3 + bias.
                y3 = y_pool.tile([P, T], mybir.dt.float32, tag="y3")
                nc.scalar.activation(
                    y3[:],
                    xt[:, K - 1 : K - 1 + T],
                    mybir.ActivationFunctionType.Identity,
                    bias=bias_sb[:],
                    scale=w_sb[:, 3:4],
                )
                t1 = y_pool.tile([P, T], mybir.dt.float32, tag="t1")

            if split2:
                # Tap 3 (+bias) on DVE: y3 = x3 * w3 + bias.
                y3 = y_pool.tile([P, T], mybir.dt.float32, tag="y3")
                nc.vector.tensor_scalar(
                    y3[:],
                    xt[:, K - 1 : K - 1 + T],
                    w_sb[:, 3:4],
                    bias_sb[:],
                    mybir.AluOpType.mult,
                    mybir.AluOpType.add,
                )

            for half in range(T // HALF):
                ps = psum_pool.tile([P, HALF], mybir.dt.float32, tag="ps")
                for q in range(HALF // TT):
                    t0 = half * HALF + q * TT
                    for k in range(pe_taps):
                        nc.tensor.matmul(
                            ps[:, q * TT : (q + 1) * TT],
                            lhs[k][:],
                            xt[:, t0 + k : t0 + k + TT],
                            start=(k == 0),
                            stop=(k == pe_taps - 1),
                        )
                h0 = half * HALF
                if v4:
                    # DVE: t1 = x1*w1 + psum(tap0), then out = x2*w2 + t1.
                    nc.vector.scalar_tensor_tensor(
                        t1[:, h0 : h0 + HALF],
                        xt[:, h0 + 1 : h0 + 1 + HALF],
                        w_sb[:, 1:2],
                        ps[:],
                        mybir.AluOpType.mult,
                        mybir.AluOpType.add,
                    )
                    nc.vector.scalar_tensor_tensor(
                        out_sb[:, h0 : h0 + HALF],
                        xt[:, h0 + 2 : h0 + 2 + HALF],
                        w_sb[:, 2:3],
                        t1[:, h0 : h0 + HALF],
                        mybir.AluOpType.mult,
                        mybir.AluOpType.add,
                    )
                elif split2:
                    # Tap 2 fused with the PSUM read on DVE:
                    #   out = x2 * w2 + psum(taps 0,1)
                    nc.vector.scalar_tensor_tensor(
                        out_sb[:, h0 : h0 + HALF],
                        xt[:, h0 + 2 : h0 + 2 + HALF],
                        w_sb[:, 2:3],
                        ps[:],
                        mybir.AluOpType.mult,
                        mybir.AluOpType.add,
                    )
                else:
                    # Evict 4 banks at once; fuse the bias add.
                    nc.scalar.activation(
                        out_sb[:, h0 : h0 + HALF],
                        ps[:],
                        mybir.ActivationFunctionType.Identity,
                        bias=bias_sb[:],
                        scale=1.0,
                    )

            if split2 or v4:
                # out += y3 on GpSimd (keeps DVE free for the PSUM MACs).
                nc.gpsimd.tensor_tensor(
                    out_sb[:], out_sb[:], y3[:], mybir.AluOpType.add
                )

            nc.sync.dma_start(o_d[c0 : c0 + P, :], out_sb[:])

    nc.compile()
    return nc


def _get_program(mm_dtype: str) -> bass.Bass:
    if mm_dtype not in _PROGRAM_CACHE:
        _PROGRAM_CACHE[mm_dtype] = _build_program(mm_dtype)
    return _PROGRAM_CACHE[mm_dtype]


def kernel(x: np.ndarray, weight: np.ndarray, bias: np.ndarray) -> np.ndarray:
    global LAST_EXEC_NS, LAST_RESULTS

    x = np.asarray(x, dtype=np.float32)
    weight = np.asarray(weight, dtype=np.float32)
    bias = np.asarray(bias, dtype=np.float32)

    # [B, T, C] -> [B, C, T] so time is contiguous per channel row.
    xt = np.ascontiguousarray(x.transpose(0, 2, 1))
    w4 = np.ascontiguousarray(weight[:, 0, :])        # [C, K]
    b2 = np.ascontiguousarray(bias.reshape(C, 1))     # [C, 1]

    nc = _get_program(MM_DTYPE)
    in_maps = [{"x": xt[b], "w": w4, "b": b2} for b in range(B)]

    trace = bool(os.environ.get("KERNEL_PROFILE"))
    if trace:
        _setup_profiling()
    res = run_bass_kernel_spmd(
        nc,
        in_maps,
        list(range(N_CORES)),
        trace=trace,
        tmpdir=os.environ.get("KERNEL_PROFILE_DIR") or None,
    )
    LAST_EXEC_NS = res.exec_time_ns
    LAST_RESULTS = res

    out = np.empty((B, T, C), dtype=np.float32)
    for b in range(B):
        out[b] = res.results[b]["out"].T
    return out


# revision 28
# speedup vs baseline: 1.5422x; 1.5422x over previous
"""Causal depthwise Conv1d (B=8, T=4096, C=2048, K=4), fp32, on 8 NeuronCores.

Strategy:
  - Batch-parallel across the 8 cores (B == 8, zero communication).
  - Host transposes x to [B, C, T] so channels land on SBUF partitions and
    time on the free dimension; then every DMA is fully contiguous and the
    4 causal taps are free-dim slices.
  - Per 128-channel block, the depthwise conv is computed on the Tensor
    engine as 4 PSUM-accumulating matmuls with diagonal weight matrices:
        psum[c, t] += diag(w_k)[c, c'] @ x[c', t - 3 + k]
    The diagonal lhsT tiles are built on-chip from a Const identity matrix
    scaled per-partition by the weights (vector tensor_scalar).
  - PSUM -> SBUF eviction with the per-channel bias fused (scalar engine
    activation Identity with bias AP), then contiguous DMA store.
  - Host transposes the [B, C, T] result back to [B, T, C].
"""

import os
from contextlib import ExitStack

import numpy as np

import concourse.bacc as bacc
import concourse.bass as bass
import concourse.mybir as mybir
import concourse.tile as tile
from concourse.bass_utils import run_bass_kernel_spmd

B, T, C, K = 8, 4096, 2048, 4
P = 128                 # partitions per channel block
CB = C // P             # 16 channel blocks
TT = 512                # moving-dim (free) tile per matmul / PSUM bank
HALF = 2048             # free elements per PSUM tile (4 banks)
N_CORES = 8

# "fp32" = exact (4 cyc/row), "fp32r" = fast PE mode (1 cyc/row, reduced
# internal precision).  Overridable via env for A/B testing.
MM_DTYPE = os.environ.get("KERNEL_MM_DTYPE", "fp32")

LAST_EXEC_NS = None
LAST_RESULTS = None

_PROGRAM_CACHE = {}
_PROFILING_READY = False


def _setup_profiling():
    """Register the axon NTFF profile hook (the image lacks
    antenv.axon_hooks, so shim it into sys.modules) and neuter the S3
    artifact upload."""
    global _PROFILING_READY
    if _PROFILING_READY:
        return
    import sys
    import types

    if "antenv.axon_hooks" not in sys.modules:
        mod = types.ModuleType("antenv.axon_hooks")
        mod._hook = None

        def set_axon_ntff_profile_hook(h):
            mod._hook = h

        def get_axon_ntff_profile_hook():
            return mod._hook

        mod.set_axon_ntff_profile_hook = set_axon_ntff_profile_hook
        mod.get_axon_ntff_profile_hook = get_axon_ntff_profile_hook
        sys.modules["antenv.axon_hooks"] = mod
        import antenv

        antenv.axon_hooks = mod

    from antenv.axon_hooks import (
        get_axon_ntff_profile_hook,
        set_axon_ntff_profile_hook,
    )

    if get_axon_ntff_profile_hook() is None:
        from trn_agent_boot.trn_boot import _ntff_profile_via_ctypes

        set_axon_ntff_profile_hook(
            _ntff_profile_via_ctypes("/opt/axon/libaxon_pjrt.so")
        )

    import concourse.bass_utils as bu

    bu.upload_artifacts = lambda tmpdir: str(tmpdir)
    _PROFILING_READY = True


def _build_program(mm_dtype: str) -> bass.Bass:
    nc = bacc.Bacc("TRN2", target_bir_lowering=False, debug=False)

    mmdt = (
        mybir.dt.float32r
        if mm_dtype in ("fp32r", "v6")
        else mybir.dt.float32
    )

    x_d = nc.dram_tensor("x", [C, T], mmdt, kind="ExternalInput")
    w_d = nc.dram_tensor("w", [C, K], mybir.dt.float32, kind="ExternalInput")
    b_d = nc.dram_tensor("b", [C, 1], mybir.dt.float32, kind="ExternalInput")
    o_d = nc.dram_tensor("out", [C, T], mybir.dt.float32, kind="ExternalOutput")
    ident_d = nc.inline_tensor(np.eye(P, dtype=np.float32), "ident")

    with tile.TileContext(nc) as tc, ExitStack() as ctx:
        id_pool = ctx.enter_context(tc.tile_pool(name="id", bufs=1))
        x_pool = ctx.enter_context(tc.tile_pool(name="x", bufs=2))
        out_pool = ctx.enter_context(tc.tile_pool(name="o", bufs=2))
        wb_pool = ctx.enter_context(tc.tile_pool(name="wb", bufs=2))
        lhs_pool = ctx.enter_context(tc.tile_pool(name="lhs", bufs=8))
        y_pool = ctx.enter_context(tc.tile_pool(name="y", bufs=2))
        psum_pool = ctx.enter_context(
            tc.tile_pool(name="ps", bufs=2, space="PSUM")
        )

        id_sb = id_pool.tile([P, P], mybir.dt.float32, tag="ident")
        nc.sync.dma_start(id_sb[:], ident_d[:])

        split2 = mm_dtype == "split2"
        v4 = mm_dtype == "v4"
        v5 = mm_dtype == "v5"
        v6 = mm_dtype == "v6"
        pe_taps = 2 if split2 else (1 if v4 else K)

        if v6:
            # fp32r taps {0,1,2} on PE (k-outer, PSUM-accumulated), tap 3 +
            # bias on ACT's free affine, PSUM+y3 combine on DVE.  Inputs and
            # outputs on different HWDGE queues (sync vs scalar) so block
            # i+1's load is not FIFO-blocked behind block i's store.
            f32r = mybir.dt.float32r
            for cb in range(CB):
                c0 = cb * P

                w_sb = wb_pool.tile([P, K], mybir.dt.float32, tag="w")
                nc.gpsimd.dma_start(w_sb[:], w_d[c0 : c0 + P, :])
                bias_sb = wb_pool.tile([P, 1], mybir.dt.float32, tag="bias")
                nc.gpsimd.dma_start(bias_sb[:], b_d[c0 : c0 + P, :])

                xt = x_pool.tile([P, T + K - 1], f32r, tag="x")
                nc.vector.memset(xt[:, 0 : K - 1].bitcast(mybir.dt.uint32), 0)
                nc.sync.dma_start(xt[:, K - 1 : T + K - 1], x_d[c0 : c0 + P, :])

                lhs = []
                for k in range(3):
                    lk = lhs_pool.tile([P, P], f32r, tag="lhs")
                    nc.scalar.mul(lk[:], id_sb[:], w_sb[:, k : k + 1])
                    lhs.append(lk)

                y3 = y_pool.tile([P, T], mybir.dt.float32, tag="y3")
                nc.scalar.activation(
                    y3[:],
                    xt[:, K - 1 : K - 1 + T].bitcast(mybir.dt.float32),
                    mybir.ActivationFunctionType.Identity,
                    bias=bias_sb[:],
                    scale=w_sb[:, 3:4],
                )

                out_sb = out_pool.tile([P, T], mybir.dt.float32, tag="o")
                for half in range(T // HALF):
                    ps = psum_pool.tile([P, HALF], mybir.dt.float32, tag="ps")
                    h0 = half * HALF
                    for k in range(3):
                        for q in range(HALF // TT):
                            t0 = h0 + q * TT
                            nc.tensor.matmul(
                                ps[:, q * TT : (q + 1) * TT],
                                lhs[k][:],
                                xt[:, t0 + k : t0 + k + TT],
                                start=(k == 0),
                                stop=(k == 2),
                                skip_group_check=True,
                            )
                    nc.vector.tensor_tensor(
                        out_sb[:, h0 : h0 + HALF],
                        ps[:],
                        y3[:, h0 : h0 + HALF],
                        mybir.AluOpType.add,
                    )
                nc.scalar.dma_start(o_d[c0 : c0 + P, :], out_sb[:])

        if v5:
            # Pure vector-engine pipeline (no PE, no PSUM), exact fp32:
            #   y0 = x0*w0 + bias     (ACT affine)
            #   t  = x1*w1 + y0       (DVE fused MAC)
            #   t  = x2*w2 + t        (DVE fused MAC, in place)
            #   y3 = x3*w3            (ACT affine)
            #   out = t + y3          (GpSimd add)
            for cb in range(CB):
                c0 = cb * P

                w_sb = wb_pool.tile([P, K], mybir.dt.float32, tag="w")
                nc.gpsimd.dma_start(w_sb[:], w_d[c0 : c0 + P, :])
                bias_sb = wb_pool.tile([P, 1], mybir.dt.float32, tag="bias")
                nc.gpsimd.dma_start(bias_sb[:], b_d[c0 : c0 + P, :])

                xt = x_pool.tile([P, T + K - 1], mybir.dt.float32, tag="x")
                nc.vector.memset(xt[:, 0 : K - 1].bitcast(mybir.dt.uint32), 0)
                nc.sync.dma_start(xt[:, K - 1 : T + K - 1], x_d[c0 : c0 + P, :])

                y0 = y_pool.tile([P, T], mybir.dt.float32, tag="y0")
                nc.scalar.activation(
                    y0[:],
                    xt[:, 0:T],
                    mybir.ActivationFunctionType.Identity,
                    bias=bias_sb[:],
                    scale=w_sb[:, 0:1],
                )
                y3 = y_pool.tile([P, T], mybir.dt.float32, tag="y3")
                nc.scalar.activation(
                    y3[:],
                    xt[:, K - 1 : K - 1 + T],
                    mybir.ActivationFunctionType.Identity,
                    bias=0.0,
                    scale=w_sb[:, 3:4],
                )
                out_sb = out_pool.tile([P, T], mybir.dt.float32, tag="o")
                for half in range(T // HALF):
                    h0 = half * HALF
                    sl = slice(h0, h0 + HALF)
                    nc.vector.scalar_tensor_tensor(
                        y0[:, sl],
                        xt[:, h0 + 1 : h0 + 1 + HALF],
                        w_sb[:, 1:2],
                        y0[:, sl],
                        mybir.AluOpType.mult,
                        mybir.AluOpType.add,
                    )
                    nc.vector.scalar_tensor_tensor(
                        y0[:, sl],
                        xt[:, h0 + 2 : h0 + 2 + HALF],
                        w_sb[:, 2:3],
                        y0[:, sl],
                        mybir.AluOpType.mult,
                        mybir.AluOpType.add,
                    )
                    nc.gpsimd.tensor_tensor(
                        out_sb[:, sl], y0[:, sl], y3[:, sl], mybir.AluOpType.add
                    )
                nc.sync.dma_start(o_d[c0 : c0 + P, :], out_sb[:])

        for cb in range(0 if (v5 or v6) else CB):
            c0 = cb * P

            w_sb = wb_pool.tile([P, K], mybir.dt.float32, tag="w")
            nc.gpsimd.dma_start(w_sb[:], w_d[c0 : c0 + P, :])
            bias_sb = wb_pool.tile([P, 1], mybir.dt.float32, tag="bias")
            nc.gpsimd.dma_start(bias_sb[:], b_d[c0 : c0 + P, :])

            # x tile with K-1 left halo columns (zeros: causal padding).
            xt = x_pool.tile([P, T + K - 1], mmdt, tag="x")
            nc.vector.memset(xt[:, 0 : K - 1].bitcast(mybir.dt.uint32), 0)
            nc.sync.dma_start(xt[:, K - 1 : T + K - 1], x_d[c0 : c0 + P, :])

            # lhsT_k = diag(w[:, k]) built as identity * w_k (per-partition).
            lhs = []
            for k in range(pe_taps):
                lk = lhs_pool.tile([P, P], mmdt, tag="lhs")
                nc.scalar.mul(lk[:], id_sb[:], w_sb[:, k : k + 1])
                lhs.append(lk)

            out_sb = out_pool.tile([P, T], mybir.dt.float32, tag="o")

            if v4:
                # Tap 3 + bias on ACT via its free affine: y3 = x3*w3 + bias.
                y3 = y_pool.tile([P, T], mybir.dt.float32, tag="y3")
                nc.scalar.activation(
                    y3[:],
                    xt[:, K - 1 : K - 1 + T],
                    mybir.ActivationFunctionType.Identity,
                    bias=bias_sb[:],
                    scale=w_sb[:, 3:4],
                )
                t1 = y_pool.tile([P, T], mybir.dt.float32, tag="t1")

            if split2:
                # Tap 3 (+bias) on DVE: y3 = x3 * w3 + bias.
                y3 = y_pool.tile([P, T], mybir.dt.float32, tag="y3")
                nc.vector.tensor_scalar(
                    y3[:],
                    xt[:, K - 1 : K - 1 + T],
                    w_sb[:, 3:4],
                    bias_sb[:],
                    mybir.AluOpType.mult,
                    mybir.AluOpType.add,
                )

            for half in range(T // HALF):
                ps = psum_pool.tile([P, HALF], mybir.dt.float32, tag="ps")
                for q in range(HALF // TT):
                    t0 = half * HALF + q * TT
                    for k in range(pe_taps):
                        nc.tensor.matmul(
                            ps[:, q * TT : (q + 1) * TT],
                            lhs[k][:],
                            xt[:, t0 + k : t0 + k + TT],
                            start=(k == 0),
                            stop=(k == pe_taps - 1),
                        )
                h0 = half * HALF
                if v4:
                    # DVE: t1 = x1*w1 + psum(tap0), then out = x2*w2 + t1.
                    nc.vector.scalar_tensor_tensor(
                        t1[:, h0 : h0 + HALF],
                        xt[:, h0 + 1 : h0 + 1 + HALF],
                        w_sb[:, 1:2],
                        ps[:],
                        mybir.AluOpType.mult,
                        mybir.AluOpType.add,
                    )
                    nc.vector.scalar_tensor_tensor(
                        out_sb[:, h0 : h0 + HALF],
                        xt[:, h0 + 2 : h0 + 2 + HALF],
                        w_sb[:, 2:3],
                        t1[:, h0 : h0 + HALF],
                        mybir.AluOpType.mult,
                        mybir.AluOpType.add,
                    )
                elif split2:
                    # Tap 2 fused with the PSUM read on DVE:
                    #   out = x2 * w2 + psum(taps 0,1)
                    nc.vector.scalar_tensor_tensor(
                        out_sb[:, h0 : h0 + HALF],
                        xt[:, h0 + 2 : h0 + 2 + HALF],
                        w_sb[:, 2:3],
                        ps[:],
                        mybir.AluOpType.mult,
                        mybir.AluOpType.add,
                    )
                else:
                    # Evict 4 banks at once; fuse the bias add.
                    nc.scalar.activation(
                        out_sb[:, h0 : h0 + HALF],
                        ps[:],
                        mybir.ActivationFunctionType.Identity,
                        bias=bias_sb[:],
                        scale=1.0,
                    )

            if split2 or v4:
                # out += y3 on GpSimd (keeps DVE free for the PSUM MACs).
                nc.gpsimd.tensor_tensor(
                    out_sb[:], out_sb[:], y3[:], mybir.AluOpType.add
                )

            nc.sync.dma_start(o_d[c0 : c0 + P, :], out_sb[:])

    nc.compile()
    return nc


def _get_program(mm_dtype: str) -> bass.Bass:
    if mm_dtype not in _PROGRAM_CACHE:
        _PROGRAM_CACHE[mm_dtype] = _build_program(mm_dtype)
    return _PROGRAM_CACHE[mm_dtype]


def kernel(x: np.ndarray, weight: np.ndarray, bias: np.ndarray) -> np.ndarray:
    global LAST_EXEC_NS, LAST_RESULTS

    x = np.asarray(x, dtype=np.float32)
    weight = np.asarray(weight, dtype=np.float32)
    bias = np.asarray(bias, dtype=np.float32)

    # [B, T, C] -> [B, C, T] so time is contiguous per channel row.
    xt = np.ascontiguousarray(x.transpose(0, 2, 1))
    w4 = np.ascontiguousarray(weight[:, 0, :])        # [C, K]
    b2 = np.ascontiguousarray(bias.reshape(C, 1))     # [C, 1]

    nc = _get_program(MM_DTYPE)
    in_maps = [{"x": xt[b], "w": w4, "b": b2} for b in range(B)]

    trace = bool(os.environ.get("KERNEL_PROFILE"))
    if trace:
        _setup_profiling()
    res = run_bass_kernel_spmd(
        nc,
        in_maps,
        list(range(N_CORES)),
        trace=trace,
        tmpdir=os.environ.get("KERNEL_PROFILE_DIR") or None,
    )
    LAST_EXEC_NS = res.exec_time_ns
    LAST_RESULTS = res

    out = np.empty((B, T, C), dtype=np.float32)
    for b in range(B):
        out[b] = res.results[b]["out"].T
    return out
